# revision 15
# baseline (speedup 1.0000x reference)
"""Trainium2 Bass kernel for ConvTranspose4d (T: 3-tap valid conv; D/H/W:
stride-2 k=3 p=1 transposed conv). Self-contained: hardcoded shapes.

x: [1, 8, 8, 24, 48, 48] f32, weight: [8, 8, 3, 3, 3, 3] f32
out: [1, 8, 6, 47, 95, 95] f32

Strategy (8 NeuronCores, data-parallel over D):
  - Core j computes output od = 6j..6j+5 (core 7 drops od 47); needs input
    D slices id0..id0+3 (id0 = min(3j, 20)).
  - Temporal 3-tap conv and the D-axis stride-2 transposed conv fold into
    the matmul stationary operand as a banded weight matrix:
      lhsT[K=128=(slot4, cin8, id4), M=96=(fbit2 x (cout8*6+od))]
    where frame pair i uses planes 2i..2i+3 at slots 0..3 and frame
    f0+fbit reads slot kt+fbit for temporal tap kt.
  - All input prefetched up front on the HWDGE (sync) queue in FIFO
    priority order: bands, xt0 rows 0-20, xt0 rest, xt1, xt2; x-tiles
    host-padded to 49x49 and pre-cast bf16 so loads are contiguous.
  - 6 warmup matmuls on a never-written scratch tile run during the
    loads (no data deps) to lift the PE HAM clock gate to 2.4 GHz.
  - H/W parities are 4 output classes (ph, pw); each accumulates 1/2/2/4
    shifted-view taps (kh = ph - 2*dh + 1) in PSUM; pw=1 classes stream
    only the 47 kept w columns.
  - Loop order: H chunk-group outer, class inner; after each group's 4
    classes are interleave-copied (DVE/ACT split) into the bf16 staging
    tile, that group's output rows are stored (3 stores per frame pair),
    so stores start early and the kernel tail only drains a 0.27MB store.
"""
import numpy as np

TAPS = {
    (0, 0): [(0, 0)],
    (0, 1): [(0, 0), (0, 1)],
    (1, 0): [(0, 0), (1, 0)],
    (1, 1): [(0, 0), (0, 1), (1, 0), (1, 1)],
}
TAP_LIST = [(ph, pw, dh, dw) for (ph, pw), tl in TAPS.items() for (dh, dw) in tl]
CHUNK_START = [0, 10, 20, 30, 40]
CHUNK_N = [10, 10, 10, 10, 8]
# (chunks, oh_start, oh_end): stores issued per group
GROUPS = [((0, 1), 0, 40), ((2, 3), 40, 80), ((4,), 80, 95)]

_CACHE = {}


def _build_bands(W, j):
    """W: [cin8, cout8, kt3, kd3, kh3, kw3] -> [128, 9, 96] f32.
    K row = slot*32 + cin*4 + id (slot = kt + fbit);
    M col = fbit*48 + cout*6 + od (od 0..5)."""
    id0 = min(3 * j, 20)
    B = np.zeros((128, 9, 96), np.float32)
    ci = np.arange(8)
    co = np.arange(8)
    for t, (ph, pw, dh, dw) in enumerate(TAP_LIST):
        kh = ph - 2 * dh + 1
        kw = pw - 2 * dw + 1
        for fbit in range(2):
            for kt in range(3):
                slot = kt + fbit
                for idl in range(4):
                    for od in range(6):
                        od_g = 6 * j + od
                        if od_g > 46:
                            continue
                        kd = od_g - 2 * (id0 + idl) + 1
                        if not (0 <= kd <= 2):
                            continue
                        krow = slot * 32 + ci * 4 + idl
                        mcol = fbit * 48 + co * 6 + od
                        B[krow[:, None], t, mcol[None, :]] = W[:, :, kt, kd, kh, kw]
    return B


def _free_view(base, off, dims):
    """Hand-built AP: keep base's partition dim, replace free dims with
    [(step, count), ...] (element units) at extra offset `off`."""
    a = base.copy()
    v = a.ap
    part = v.to_list()[0]
    v.clear()
    v.append(part)
    for sc in dims:
        v.append(list(sc))
    a.ap = v
    a.offset = a.offset + off
    return a


def _build_program():
    import concourse.bacc as bacc
    import concourse.tile as tile
    from concourse import mybir

    f32 = mybir.dt.float32
    bf16 = mybir.dt.bfloat16

    nc = bacc.Bacc("TRN2", target_bir_lowering=False, debug=False)
    # xs padded+cast on host: [t8, c8, i4, 49, 49] bf16
    xs_ap = nc.dram_tensor("xs", [8, 8, 4, 49, 49], bf16, kind="ExternalInput").ap()
    bd_ap = nc.dram_tensor("bands", [128, 9, 96], bf16, kind="ExternalInput").ap()
    # out: [f6, c8, od6, oh*ow] bf16
    out_ap = nc.dram_tensor("out", [6, 8, 6, 9025], bf16, kind="ExternalOutput").ap()

    SPLIT = 21 * 49  # xt0 early rows: chunk-group A reads rows <= 20

    with tile.TileContext(nc, trace_sim=False) as tc:
        with (
            tc.tile_pool(name="xp", bufs=1) as xp,
            tc.tile_pool(name="bp", bufs=1) as bp,
            tc.tile_pool(name="wp", bufs=1) as wp,
            tc.tile_pool(name="sp", bufs=1) as sp,
            tc.tile_pool(name="psA", bufs=4, space="PSUM") as psA,
        ):
            bt = bp.tile([128, 9, 96], bf16)
            nc.sync.dma_start(out=bt[:], in_=bd_ap)
            xts = []
            for i in range(3):
                xt = xp.tile([128, 2401], bf16, name=f"xt{i}", tag=f"xt{i}")
                src = xs_ap[2 * i:2 * i + 4].rearrange("p c i h w -> (p c i) (h w)")
                if i == 0:
                    nc.sync.dma_start(out=xt[:, 0:SPLIT], in_=src[:, 0:SPLIT])
                    nc.sync.dma_start(out=xt[:, SPLIT:2401], in_=src[:, SPLIT:2401])
                else:
                    nc.sync.dma_start(out=xt[:], in_=src)
                xts.append(xt)

            # HAM pre-warm: dense PE activity during the input loads.
            # The scratch tile is memset (no input deps) so the warmup
            # matmuls start right after the engine preamble.
            scratch = wp.tile([128, 512], bf16)
            nc.vector.memset(scratch[:], 0.0)
            for _ in range(7):
                wt = psA.tile([128, 1024], f32, name="pw", tag="pa")
                nc.tensor.matmul(
                    wt[0:96, 0:480], scratch[:, 0:96], scratch[:, 0:480],
                    start=True, stop=True,
                )

            # group-major order: the kernel ends on the three small C-group
            # stores (0.27MB each) instead of one pair's full 1.73MB
            stgs = [
                sp.tile([96, 9025], bf16, name=f"stg{i}", tag=f"stg{i}")
                for i in range(3)
            ]
            for (pair, oh0, oh1) in GROUPS:
                for i in range(3):
                    xt = xts[i]
                    stg = stgs[i]
                    for (ph, pw) in TAPS:
                        taps = TAPS[(ph, pw)]
                        nmw = 48 - pw
                        pt = psA.tile([128, 1024], f32, name="pa", tag="pa")
                        for ci_, c in enumerate(pair):
                            mh0 = CHUNK_START[c]
                            nmh = CHUNK_N[c] - (ph if c == 4 else 0)
                            for ti, (dh, dw) in enumerate(taps):
                                t_idx = TAP_LIST.index((ph, pw, dh, dw))
                                lhsT = bt[:, t_idx, :]
                                rhs = _free_view(
                                    xt, (mh0 + dh) * 49 + dw, [(49, nmh), (1, nmw)]
                                )
                                outp = pt[0:96, ci_ * 512:ci_ * 512 + nmh * nmw]
                                nc.tensor.matmul(
                                    outp, lhsT, rhs,
                                    start=(ti == 0), stop=(ti == len(taps) - 1),
                                )
                        # interleave-copy PSUM -> staging (f32 -> bf16)
                        use_scalar = ph == 1 and (pw == 1 or len(pair) == 2)
                        psrc = pt[0:96]
                        if len(pair) == 2:
                            nmh = 10
                            src = _free_view(psrc, 0, [(512, 2), (nmw, nmh), (1, nmw)])
                            doff = (2 * CHUNK_START[pair[0]] + ph) * 95 + pw
                            dst = _free_view(stg, doff, [(1900, 2), (190, nmh), (2, nmw)])
                        else:
                            nmh = CHUNK_N[4] - ph
                            src = _free_view(psrc, 0, [(nmw, nmh), (1, nmw)])
                            doff = (2 * CHUNK_START[4] + ph) * 95 + pw
                            dst = _free_view(stg, doff, [(190, nmh), (2, nmw)])
                        if use_scalar:
                            nc.scalar.copy(dst, src)
                        else:
                            nc.vector.tensor_copy(dst, src)
                    # C-group stores ride the idle gpsimd SWDGE ring so the
                    # kernel tail doesn't FIFO-serialize behind the B stores
                    deng = nc.gpsimd if oh0 == 80 else nc.sync
                    deng.dma_start(
                        out=out_ap[2 * i:2 * i + 2]
                        .rearrange("f c o x -> (f c o) x")[:, oh0 * 95:oh1 * 95],
                        in_=stg[:, oh0 * 95:oh1 * 95],
                    )

    nc.compile()
    return nc


def _get_program():
    if "nc" not in _CACHE:
        _CACHE["nc"] = _build_program()
    return _CACHE["nc"]


def _host_inputs(x, weight):
    import ml_dtypes

    x = np.asarray(x, dtype=np.float32)
    weight = np.asarray(weight, dtype=np.float32)
    xt_ = x[0].transpose(1, 0, 2, 3, 4)  # [t, c, d, h, w]
    in_maps = []
    for j in range(8):
        id0 = min(3 * j, 20)
        xs = np.zeros((8, 8, 4, 49, 49), np.float32)
        xs[:, :, :, :48, :48] = xt_[:, :, id0:id0 + 4]
        in_maps.append({
            "xs": xs.astype(ml_dtypes.bfloat16),
            "bands": _build_bands(weight, j).astype(ml_dtypes.bfloat16),
        })
    return in_maps


def run(x, weight, trace=False):
    from concourse.bass_utils import run_bass_kernel_spmd

    in_maps = _host_inputs(x, weight)
    nc = _get_program()
    res = run_bass_kernel_spmd(nc, in_maps, core_ids=list(range(8)), trace=trace)
    full = np.zeros((1, 8, 6, 47, 95, 95), np.float32)
    for j in range(8):
        nod = min(6, 47 - 6 * j)
        oj = np.asarray(res.results[j]["out"], dtype=np.float32)  # [6, 8, 6, 9025]
        oj = oj.transpose(1, 0, 2, 3).reshape(8, 6, 6, 95, 95)
        full[0, :, :, 6 * j:6 * j + nod] = oj[:, :, :nod]
    return full, res


def kernel(x, weight):
    return run(x, weight)[0]


# revision 16
# speedup vs baseline: 1.0194x; 1.0194x over previous
"""Trainium2 Bass kernel for ConvTranspose4d (T: 3-tap valid conv; D/H/W:
stride-2 k=3 p=1 transposed conv). Self-contained: hardcoded shapes.

x: [1, 8, 8, 24, 48, 48] f32, weight: [8, 8, 3, 3, 3, 3] f32
out: [1, 8, 6, 47, 95, 95] f32

Strategy (8 NeuronCores, data-parallel over D):
  - Core j computes output od = 6j..6j+5 (core 7 drops od 47); needs input
    D slices id0..id0+3 (id0 = min(3j, 20)).
  - Temporal 3-tap conv and the D-axis stride-2 transposed conv fold into
    the matmul stationary operand as a banded weight matrix:
      lhsT[K=128=(slot4, cin8, id4), M=96=(fbit2 x (cout8*6+od))]
    where frame pair i uses planes 2i..2i+3 at slots 0..3 and frame
    f0+fbit reads slot kt+fbit for temporal tap kt.
  - All input prefetched up front on the HWDGE (sync) queue in FIFO
    priority order: bands, xt0 rows 0-20, xt0 rest, xt1, xt2; x-tiles
    host-padded to 49x49 and pre-cast bf16 so loads are contiguous.
  - 6 warmup matmuls on a never-written scratch tile run during the
    loads (no data deps) to lift the PE HAM clock gate to 2.4 GHz.
  - H/W parities are 4 output classes (ph, pw); each accumulates 1/2/2/4
    shifted-view taps (kh = ph - 2*dh + 1) in PSUM; pw=1 classes stream
    only the 47 kept w columns.
  - Loop order: H chunk-group outer, class inner; after each group's 4
    classes are interleave-copied (DVE/ACT split) into the bf16 staging
    tile, that group's output rows are stored (3 stores per frame pair),
    so stores start early and the kernel tail only drains a 0.27MB store.
"""
import numpy as np

TAPS = {
    (0, 0): [(0, 0)],
    (0, 1): [(0, 0), (0, 1)],
    (1, 0): [(0, 0), (1, 0)],
    (1, 1): [(0, 0), (0, 1), (1, 0), (1, 1)],
}
TAP_LIST = [(ph, pw, dh, dw) for (ph, pw), tl in TAPS.items() for (dh, dw) in tl]
CHUNK_START = [0, 10, 20, 30, 40]
CHUNK_N = [10, 10, 10, 10, 8]
# (chunks, oh_start, oh_end): stores issued per group
GROUPS = [((0, 1), 0, 40), ((2, 3), 40, 80), ((4,), 80, 95)]

_CACHE = {}


def _build_bands(W, j):
    """W: [cin8, cout8, kt3, kd3, kh3, kw3] -> [128, 9, 96] f32.
    K row = slot*32 + cin*4 + id (slot = kt + fbit);
    M col = fbit*48 + cout*6 + od (od 0..5)."""
    id0 = min(3 * j, 20)
    B = np.zeros((128, 9, 96), np.float32)
    ci = np.arange(8)
    co = np.arange(8)
    for t, (ph, pw, dh, dw) in enumerate(TAP_LIST):
        kh = ph - 2 * dh + 1
        kw = pw - 2 * dw + 1
        for fbit in range(2):
            for kt in range(3):
                slot = kt + fbit
                for idl in range(4):
                    for od in range(6):
                        od_g = 6 * j + od
                        if od_g > 46:
                            continue
                        kd = od_g - 2 * (id0 + idl) + 1
                        if not (0 <= kd <= 2):
                            continue
                        krow = slot * 32 + ci * 4 + idl
                        mcol = fbit * 48 + co * 6 + od
                        B[krow[:, None], t, mcol[None, :]] = W[:, :, kt, kd, kh, kw]
    return B


def _free_view(base, off, dims):
    """Hand-built AP: keep base's partition dim, replace free dims with
    [(step, count), ...] (element units) at extra offset `off`."""
    a = base.copy()
    v = a.ap
    part = v.to_list()[0]
    v.clear()
    v.append(part)
    for sc in dims:
        v.append(list(sc))
    a.ap = v
    a.offset = a.offset + off
    return a


def _build_program():
    import concourse.bacc as bacc
    import concourse.tile as tile
    from concourse import mybir

    f32 = mybir.dt.float32
    bf16 = mybir.dt.bfloat16

    nc = bacc.Bacc("TRN2", target_bir_lowering=False, debug=False)
    # xs padded+cast on host: [t8, c8, i4, 49, 49] bf16
    xs_ap = nc.dram_tensor("xs", [8, 8, 4, 49, 49], bf16, kind="ExternalInput").ap()
    bd_ap = nc.dram_tensor("bands", [128, 9, 96], bf16, kind="ExternalInput").ap()
    # out: [f6, c8, od6, oh*ow] bf16
    out_ap = nc.dram_tensor("out", [6, 8, 6, 9025], bf16, kind="ExternalOutput").ap()

    SPLIT = 21 * 49  # xt0 early rows: chunk-group A reads rows <= 20

    with tile.TileContext(nc, trace_sim=False) as tc:
        with (
            tc.tile_pool(name="xp", bufs=1) as xp,
            tc.tile_pool(name="bp", bufs=1) as bp,
            tc.tile_pool(name="wp", bufs=1) as wp,
            tc.tile_pool(name="sp", bufs=1) as sp,
            tc.tile_pool(name="psA", bufs=4, space="PSUM") as psA,
        ):
            bt = bp.tile([128, 9, 96], bf16)
            nc.sync.dma_start(out=bt[:], in_=bd_ap)
            xts = []
            for i in range(3):
                xt = xp.tile([128, 2401], bf16, name=f"xt{i}", tag=f"xt{i}")
                src = xs_ap[2 * i:2 * i + 4].rearrange("p c i h w -> (p c i) (h w)")
                if i == 0:
                    nc.sync.dma_start(out=xt[:, 0:SPLIT], in_=src[:, 0:SPLIT])
                    nc.sync.dma_start(out=xt[:, SPLIT:2401], in_=src[:, SPLIT:2401])
                else:
                    nc.sync.dma_start(out=xt[:], in_=src)
                xts.append(xt)

            # HAM pre-warm: dense PE activity during the input loads.
            # The scratch tile is memset (no input deps) so the warmup
            # matmuls start right after the engine preamble.
            scratch = wp.tile([128, 512], bf16)
            nc.vector.memset(scratch[:], 0.0)
            for _ in range(7):
                wt = psA.tile([128, 1024], f32, name="pw", tag="pa")
                nc.tensor.matmul(
                    wt[0:96, 0:480], scratch[:, 0:96], scratch[:, 0:480],
                    start=True, stop=True,
                )

            # group-major order: the kernel ends on the three small C-group
            # stores (0.27MB each) instead of one pair's full 1.73MB
            stgs = [
                sp.tile([96, 9025], bf16, name=f"stg{i}", tag=f"stg{i}")
                for i in range(3)
            ]
            for (pair, oh0, oh1) in GROUPS:
                for i in range(3):
                    xt = xts[i]
                    stg = stgs[i]
                    for (ph, pw) in TAPS:
                        taps = TAPS[(ph, pw)]
                        nmw = 48 - pw
                        pt = psA.tile([128, 1024], f32, name="pa", tag="pa")
                        for ci_, c in enumerate(pair):
                            mh0 = CHUNK_START[c]
                            nmh = CHUNK_N[c] - (ph if c == 4 else 0)
                            for ti, (dh, dw) in enumerate(taps):
                                t_idx = TAP_LIST.index((ph, pw, dh, dw))
                                lhsT = bt[:, t_idx, :]
                                rhs = _free_view(
                                    xt, (mh0 + dh) * 49 + dw, [(49, nmh), (1, nmw)]
                                )
                                outp = pt[0:96, ci_ * 512:ci_ * 512 + nmh * nmw]
                                nc.tensor.matmul(
                                    outp, lhsT, rhs,
                                    start=(ti == 0), stop=(ti == len(taps) - 1),
                                )
                        # interleave-copy PSUM -> staging (f32 -> bf16)
                        use_scalar = ph == 1 and (pw == 1 or len(pair) == 2)
                        psrc = pt[0:96]
                        if len(pair) == 2:
                            nmh = 10
                            src = _free_view(psrc, 0, [(512, 2), (nmw, nmh), (1, nmw)])
                            doff = (2 * CHUNK_START[pair[0]] + ph) * 95 + pw
                            dst = _free_view(stg, doff, [(1900, 2), (190, nmh), (2, nmw)])
                        else:
                            nmh = CHUNK_N[4] - ph
                            src = _free_view(psrc, 0, [(nmw, nmh), (1, nmw)])
                            doff = (2 * CHUNK_START[4] + ph) * 95 + pw
                            dst = _free_view(stg, doff, [(190, nmh), (2, nmw)])
                        if use_scalar:
                            nc.scalar.copy(dst, src)
                        else:
                            nc.vector.tensor_copy(dst, src)
                    nc.sync.dma_start(
                        out=out_ap[2 * i:2 * i + 2]
                        .rearrange("f c o x -> (f c o) x")[:, oh0 * 95:oh1 * 95],
                        in_=stg[:, oh0 * 95:oh1 * 95],
                    )

    nc.compile()
    return nc


def _get_program():
    if "nc" not in _CACHE:
        _CACHE["nc"] = _build_program()
    return _CACHE["nc"]


def _host_inputs(x, weight):
    import ml_dtypes

    x = np.asarray(x, dtype=np.float32)
    weight = np.asarray(weight, dtype=np.float32)
    xt_ = x[0].transpose(1, 0, 2, 3, 4)  # [t, c, d, h, w]
    in_maps = []
    for j in range(8):
        id0 = min(3 * j, 20)
        xs = np.zeros((8, 8, 4, 49, 49), np.float32)
        xs[:, :, :, :48, :48] = xt_[:, :, id0:id0 + 4]
        in_maps.append({
            "xs": xs.astype(ml_dtypes.bfloat16),
            "bands": _build_bands(weight, j).astype(ml_dtypes.bfloat16),
        })
    return in_maps


def run(x, weight, trace=False):
    from concourse.bass_utils import run_bass_kernel_spmd

    in_maps = _host_inputs(x, weight)
    nc = _get_program()
    res = run_bass_kernel_spmd(nc, in_maps, core_ids=list(range(8)), trace=trace)
    full = np.zeros((1, 8, 6, 47, 95, 95), np.float32)
    for j in range(8):
        nod = min(6, 47 - 6 * j)
        oj = np.asarray(res.results[j]["out"], dtype=np.float32)  # [6, 8, 6, 9025]
        oj = oj.transpose(1, 0, 2, 3).reshape(8, 6, 6, 95, 95)
        full[0, :, :, 6 * j:6 * j + nod] = oj[:, :, :nod]
    return full, res


def kernel(x, weight):
    return run(x, weight)[0]
